# revision 73
# baseline (speedup 1.0000x reference)
"""Bahdanau attention kernel for Trainium2, data-parallel over batch on 8 NeuronCores.

Reference computation (per batch b):
    q   = query0 + query1                          # (1, H)
    wq  = q @ Wa_w.T + Wa_b                        # (1, H)
    uk  = keys @ Ua_w.T + Ua_b                     # (S, H)
    e   = tanh(wq + uk)                            # (S, H)
    s   = e @ Va_w[0] + Va_b[0]                    # (S,)
    w   = softmax(s)                               # (S,)   (Va_b shift cancels)
    ctx = w @ keys                                 # (1, H)

Shapes: B=32, S=2048, H=1024, fp32. Sharding: 4 batches per core, weights
replicated. Everything is computed on-device; the host only shards/gathers.

Device design (per core, 4 batches; keys is read from HBM exactly once):
  - Ua/Wa are transposed once on-chip via PE transposes -> UaT/WaT [h, o];
    wq = Wa@q + Wa_b + Ua_b is precomputed per (o, b) as the tanh bias.
  - Per 512-wide s-block: keys tiles load in natural layout [s, h] and are
    PE-transposed (4 per 512-wide PSUM strip, one copy out) into keysT
    [h, s] tiles — the uk contraction over h must sit on partitions.
  - uk accumulates in PSUM as ukT [o(128), s(512)] so the wq bias add and
    tanh fuse into one ScalarE activation (bias is per-partition), and the
    score dot-product runs on PE with VaT as a 1-column stationary operand.
  - Softmax is flash-attention style: each block keeps a local max m_sb and
    exp-sum z_sb (ScalarE exp with accum_out), and a PARTIAL context
    sum_s exp(score - m_sb) * keys[s] accumulates on PE while the block's
    natural keys tiles are still resident in SBUF. A cheap combine phase
    rescales blocks by exp(m_sb - M)/Z to produce weights and context.
  - Cross-engine round trips are software-pipelined: each block's score
    matmuls trail its tanh by one o-tile, each block's softmax tail trails
    by one block, and each batch's combine trails by one batch.
Matmuls run as float32r (fp32 data rounded to an 11-bit mantissa by the
producing copies; full PE rate, ~1e-4 end-to-end relative error).
"""

from contextlib import ExitStack

import numpy as np

import concourse.bass as bass
import concourse.mybir as mybir
import concourse.tile as tile
from concourse import bacc
from concourse.bass_utils import run_bass_kernel_spmd
from concourse.masks import make_identity

B, S, H = 32, 2048, 1024
NCORES = 8
BPC = B // NCORES          # batches per core
F32 = mybir.dt.float32
F32R = mybir.dt.float32r

HT = H // 128              # h tiles (contraction)
OT = H // 128              # o tiles (uk output dim)
NSB = 4                    # s-blocks of 512 per batch
SBW = S // NSB             # 512
NSS = SBW // 128           # s-subtiles per block
ST = S // 128              # 16 s-tiles per batch

_CACHE: dict = {}


def _emit(nc, tc, ctx):
    q0 = nc.dram_tensor("q0", [BPC, H], F32, kind="ExternalInput").ap()
    q1 = nc.dram_tensor("q1", [BPC, H], F32, kind="ExternalInput").ap()
    keys = nc.dram_tensor("keys", [BPC, S, H], F32, kind="ExternalInput").ap()
    wa_w = nc.dram_tensor("wa_w", [H, H], F32, kind="ExternalInput").ap()
    wa_b = nc.dram_tensor("wa_b", [H], F32, kind="ExternalInput").ap()
    ua_w = nc.dram_tensor("ua_w", [H, H], F32, kind="ExternalInput").ap()
    ua_b = nc.dram_tensor("ua_b", [H], F32, kind="ExternalInput").ap()
    va_w = nc.dram_tensor("va_w", [H], F32, kind="ExternalInput").ap()
    ctx_out = nc.dram_tensor("ctx_out", [BPC, H], F32, kind="ExternalOutput").ap()
    w_out = nc.dram_tensor("w_out", [BPC, S], F32, kind="ExternalOutput").ap()

    Tanh = mybir.ActivationFunctionType.Tanh
    Exp = mybir.ActivationFunctionType.Exp

    consts = ctx.enter_context(tc.tile_pool(name="consts", bufs=1))
    psum_tr = ctx.enter_context(tc.tile_pool(name="psum_tr", bufs=3, space="PSUM"))
    psum_uk = ctx.enter_context(tc.tile_pool(name="psum_uk", bufs=2, space="PSUM"))
    psum_sc = ctx.enter_context(tc.tile_pool(name="psum_sc", bufs=1, space="PSUM"))
    psum_ctx = ctx.enter_context(tc.tile_pool(name="psum_ctx", bufs=2, space="PSUM"))

    ident = consts.tile([128, 128], F32)
    make_identity(nc, ident[:])
    identr = consts.tile([128, 128], F32R)
    nc.vector.tensor_copy(identr[:], ident[:])

    UaT = consts.tile([128, HT, H], F32R)
    cb = consts.tile([128, OT], F32)       # Wa_b + Ua_b
    VaT = consts.tile([128, OT], F32R)
    qT = consts.tile([128, HT, BPC], F32R)
    wqcb = consts.tile([128, OT, BPC], F32)
    one1 = consts.tile([1, 1], F32)
    nc.vector.memset(one1[:], 1.0)

    knat = ctx.enter_context(tc.tile_pool(name="knat", bufs=12))
    uapool = ctx.enter_context(tc.tile_pool(name="uapool", bufs=6))
    preloaded_kns = {}
    lazy_ua = {}
    copy_tick2 = [0]

    def emit_ua_strips(ot, nat):
        for hg in range(0, HT, 4):
            ps = psum_tr.tile([128, 512], F32, tag="tr")
            for hi in range(4):
                nc.tensor.matmul(
                    ps[:, hi * 128:(hi + 1) * 128],
                    lhsT=nat[:, (hg + hi) * 128:(hg + hi + 1) * 128],
                    rhs=ident[:], is_transpose=True,
                    skip_group_check=True,
                )
            ps_v = ps.rearrange("p (h x) -> p h x", h=4)
            dst = UaT[:, hg:hg + 4, ot * 128:(ot + 1) * 128]
            if copy_tick2[0] % 2 == 0:
                nc.vector.tensor_copy(dst, ps_v)
            else:
                nc.scalar.copy(dst, ps_v)
            copy_tick2[0] += 1

    def load_kn_block(b, sb):
        kns = []
        for ss in range(NSS):
            kn = knat.tile([128, H], F32R, tag="kn")
            s0 = sb * SBW + ss * 128
            nc.sync.dma_start(out=kn[:], in_=keys[b, s0:s0 + 128, :].bitcast(F32R))
            kns.append(kn)
        return kns

    # ---- setup phase (pools released before the main loop) ----
    with ExitStack() as sctx:
        setup = sctx.enter_context(tc.tile_pool(name="setup", bufs=2))
        wpool = sctx.enter_context(tc.tile_pool(name="wpool", bufs=1))

        # small input DMAs on the ACT hwdge queue (the big weight/keys loads
        # own the SP queue, so descriptor generation overlaps)
        wab_n = setup.tile([1, H], F32, tag="bias", bufs=4)
        uab_n = setup.tile([1, H], F32, tag="bias", bufs=4)
        va_n = setup.tile([1, H], F32, tag="bias", bufs=4)
        q0s = setup.tile([BPC, H], F32, tag="qin")
        q1s = setup.tile([BPC, H], F32, tag="qin")
        qs = setup.tile([BPC, H], F32, tag="qsum")
        nc.scalar.dma_start(out=wab_n[:], in_=wa_b.rearrange("(o h) -> o h", o=1))
        nc.scalar.dma_start(out=uab_n[:], in_=ua_b.rearrange("(o h) -> o h", o=1))
        nc.scalar.dma_start(out=va_n[:], in_=va_w.rearrange("(o h) -> o h", o=1))
        nc.scalar.dma_start(out=q0s[:], in_=q0[:, :])
        nc.scalar.dma_start(out=q1s[:], in_=q1[:, :])

        # one-time weight transposes: UaT/WaT[h_part, h_tile, o].
        # Wa first (it feeds the tiny wq matmuls), then Ua (which gates the
        # main uk matmuls), then the first keys blocks. Each weight row-tile
        # is transposed as soon as its DMA lands (two 512-wide PSUM strips
        # per tile) so the PE is never idle for long during the DMA ramp.
        WaT = wpool.tile([128, HT, H], F32R)
        copy_tick = 0

        def transpose_nat(src_dram, dstT, ot):
            nonlocal copy_tick
            nat = setup.tile([128, H], F32, tag="nat", bufs=8)
            nc.sync.dma_start(out=nat[:], in_=src_dram[ot * 128:(ot + 1) * 128, :])
            for hg in range(0, HT, 4):
                ps = psum_tr.tile([128, 512], F32, tag="tr")
                for hi in range(4):
                    nc.tensor.matmul(
                        ps[:, hi * 128:(hi + 1) * 128],
                        lhsT=nat[:, (hg + hi) * 128:(hg + hi + 1) * 128],
                        rhs=ident[:], is_transpose=True,
                        skip_group_check=True,
                    )
                dst = dstT[:, hg:hg + 4, ot * 128:(ot + 1) * 128]
                ps_v = ps.rearrange("p (h x) -> p h x", h=4)
                if copy_tick % 2 == 0:
                    nc.vector.tensor_copy(dst, ps_v)
                else:
                    nc.scalar.copy(dst, ps_v)
                copy_tick += 1

        for ot in range(OT):
            transpose_nat(wa_w, WaT, ot)

        # q/bias pipelines (loads already emitted above, before the weight DMAs)
        cb_n = setup.tile([1, H], F32, tag="bias", bufs=4)
        nc.vector.tensor_add(cb_n[:], wab_n[:], uab_n[:])
        ps_cb = psum_tr.tile([128, OT], F32, tag="tr")
        ps_va = psum_tr.tile([128, OT], F32, tag="tr")
        for ht in range(HT):
            nc.tensor.matmul(
                ps_cb[:, ht:ht + 1], lhsT=cb_n[:, ht * 128:(ht + 1) * 128],
                rhs=one1[:], start=True, stop=True, skip_group_check=True,
            )
            nc.tensor.matmul(
                ps_va[:, ht:ht + 1], lhsT=va_n[:, ht * 128:(ht + 1) * 128],
                rhs=one1[:], start=True, stop=True, skip_group_check=True,
            )
        nc.vector.tensor_copy(cb[:], ps_cb[:])
        nc.vector.tensor_copy(VaT[:], ps_va[:])
        nc.vector.tensor_add(qs[:], q0s[:], q1s[:])

        # q transpose + wq matmuls (WaT-dependent, emitted right after Wa)
        for ht in range(HT):
            ps = psum_tr.tile([128, BPC], F32, tag="tr")
            nc.tensor.transpose(ps[:], qs[:, ht * 128:(ht + 1) * 128], ident[:BPC, :BPC])
            nc.vector.tensor_copy(qT[:, ht, :], ps[:])
        for ot in range(OT):
            pw = psum_tr.tile([128, BPC], F32, tag="tr")
            for ht in range(HT):
                nc.tensor.matmul(
                    pw[:],
                    lhsT=WaT[:, ht, ot * 128:(ot + 1) * 128],
                    rhs=qT[:, ht, :],
                    start=(ht == 0),
                    stop=(ht == HT - 1),
                )
            nc.vector.tensor_scalar(
                out=wqcb[:, ot, :], in0=pw[:], scalar1=cb[:, ot:ot + 1],
                scalar2=None, op0=mybir.AluOpType.add,
            )

        # Ua: first two tiles eager, the first keys block next in the DMA
        # stream, then the remaining Ua tiles load lazily — their transposes
        # are emitted inside batch 0's first uk loop, so real uk matmuls
        # overlap the tail of the weight stream
        for ot in range(2):
            transpose_nat(ua_w, UaT, ot)
        preloaded_kns[(0, 0)] = load_kn_block(0, 0)
        for ot in range(2, OT):
            nat_l = uapool.tile([128, H], F32, tag="uan", name=f"uan{ot}")
            nc.sync.dma_start(out=nat_l[:], in_=ua_w[ot * 128:(ot + 1) * 128, :])
            lazy_ua[ot] = nat_l
        preloaded_kns[(0, 1)] = load_kn_block(0, 1)

    ktp = ctx.enter_context(tc.tile_pool(name="ktp", bufs=2))
    epool = ctx.enter_context(tc.tile_pool(name="epool", bufs=3))
    sexp = ctx.enter_context(tc.tile_pool(name="sexp", bufs=7))
    pcp = ctx.enter_context(tc.tile_pool(name="pcp", bufs=6))
    small = ctx.enter_context(tc.tile_pool(name="small", bufs=4))
    ctxp = ctx.enter_context(tc.tile_pool(name="ctxp", bufs=2))

    # ---- main per-batch pipeline ----
    # Scores for each 512-wide s-block are softmaxed LOCALLY (flash-attention
    # style: per-block max m_sb and sum z_sb), and a partial context
    # Σ_s exp(score-m_sb)·keys[s] is accumulated while the keys tiles are
    # still in SBUF — so keys is read from HBM exactly once. The combine
    # phase rescales the per-block exponentials and partial contexts by
    # exp(m_sb - M)/Z with the global max/sum.
    pending_tail = [None]   # (flash_tail_fn, args) deferred across blocks/batches

    def scores_phase(b):
        """Per-block scores, local exp + partial context for batch b."""
        nonlocal copy_tick
        mstrip = small.tile([1, NSB], F32, tag="mstrip")   # -m_sb per block
        zstrip = small.tile([1, NSB], F32, tag="zstrip")   # z_sb per block
        e_sbs, pcs = [], []

        def flash_tail(sb, sc_sb, kns):
            # local softmax pieces for this block: m_sb, e_sb, z_sb
            nc.vector.tensor_reduce(
                mstrip[:, sb:sb + 1], sc_sb[:], axis=mybir.AxisListType.X,
                op=mybir.AluOpType.max, negate=True,
            )
            e_sb = sexp.tile([1, SBW], F32, tag="esb")
            nc.scalar.activation(
                e_sb[:], sc_sb[:], Exp,
                bias=mstrip[:, sb:sb + 1], accum_out=zstrip[:, sb:sb + 1],
            )
            e_sbs.append(e_sb)

            # transpose e_sb to [s_part, s_subtile] for the context matmuls
            psT = psum_tr.tile([128, NSS], F32, tag="tr")
            for ss in range(NSS):
                nc.tensor.matmul(
                    psT[:, ss:ss + 1],
                    lhsT=e_sb[:, ss * 128:(ss + 1) * 128], rhs=one1[:],
                    start=True, stop=True, skip_group_check=True,
                )
            eT = sexp.tile([128, NSS], F32R, tag="eT")
            nc.vector.tensor_copy(eT[:], psT[:])

            # partial context for this block (keys tiles still resident)
            pc_ps0 = psum_ctx.tile([1, 512], F32, tag="ctx")
            pc_ps1 = psum_ctx.tile([1, 512], F32, tag="ctx")
            for ss in range(NSS):
                nc.tensor.matmul(
                    pc_ps0[:], lhsT=eT[:, ss:ss + 1], rhs=kns[ss][:, 0:512],
                    start=(ss == 0), stop=(ss == NSS - 1),
                )
                nc.tensor.matmul(
                    pc_ps1[:], lhsT=eT[:, ss:ss + 1], rhs=kns[ss][:, 512:1024],
                    start=(ss == 0), stop=(ss == NSS - 1),
                )
            pc = pcp.tile([1, H], F32, tag="pc")
            nc.vector.tensor_copy(pc[:, 0:512], pc_ps0[:])
            nc.vector.tensor_copy(pc[:, 512:1024], pc_ps1[:])
            pcs.append(pc)

        for sb in range(NSB):
            # transpose this 512-wide block of keys: keysT[h_part, h_tile, s]
            kns = preloaded_kns.pop((b, sb), None) or load_kn_block(b, sb)
            kT = ktp.tile([128, HT, SBW], F32R, tag="kT")
            for ht in range(HT):
                ps = psum_tr.tile([128, SBW], F32R, tag="tr")
                for ss in range(NSS):
                    nc.tensor.matmul(
                        ps[:, ss * 128:(ss + 1) * 128],
                        lhsT=kns[ss][:, ht * 128:(ht + 1) * 128],
                        rhs=identr[:], is_transpose=True,
                        skip_group_check=True,
                    )
                if copy_tick % 2 == 0:
                    nc.vector.tensor_copy(kT[:, ht, :], ps[:])
                else:
                    nc.scalar.copy(kT[:, ht, :], ps[:])
                copy_tick += 1
            # ukT[o(128), s(512)] per o-tile; fused bias+tanh; score accumulation
            # (the score matmul for o-tile `ot` is emitted after the uk matmuls
            # of o-tile `ot+1`, so the PE never waits on the tanh round trip)
            sc_ps = psum_sc.tile([1, SBW], F32, tag="sc")
            prev_e = None
            for ot in range(OT):
                if ot in lazy_ua:
                    emit_ua_strips(ot, lazy_ua.pop(ot))
                uk_ps = psum_uk.tile([128, SBW], F32, tag="uk")
                for ht in range(HT):
                    nc.tensor.matmul(
                        uk_ps[:],
                        lhsT=UaT[:, ht, ot * 128:(ot + 1) * 128],
                        rhs=kT[:, ht, :],
                        start=(ht == 0),
                        stop=(ht == HT - 1),
                    )
                e_t = epool.tile([128, SBW], F32R, tag="e", bufs=4)
                nc.scalar.activation(e_t[:], uk_ps[:], Tanh, bias=wqcb[:, ot, b:b + 1])
                if prev_e is not None:
                    nc.tensor.matmul(
                        sc_ps[:], lhsT=VaT[:, ot - 1:ot], rhs=prev_e[:],
                        start=(ot == 1), stop=False,
                    )
                prev_e = e_t
                if ot == 3 and pending_tail[0] is not None:
                    # emit the previous block's flash tail now, behind dense
                    # PE work, so its latency chain hides under the uk matmuls
                    fn, args = pending_tail[0]
                    fn(*args)
                    pending_tail[0] = None
            nc.tensor.matmul(
                sc_ps[:], lhsT=VaT[:, OT - 1:OT], rhs=prev_e[:],
                start=False, stop=True,
            )
            sc_sb = epool.tile([1, SBW], F32, tag="scs", bufs=2)
            nc.vector.tensor_copy(sc_sb[:], sc_ps[:])
            pending_tail[0] = (flash_tail, (sb, sc_sb, kns))
        return mstrip, zstrip, e_sbs, pcs

    def combine_phase(b, mstrip, zstrip, e_sbs, pcs):
        """Global softmax rescale + output assembly for batch b."""
        # nmM = -M (global); d_sb = M - m_sb; alpha_sb = exp(m_sb - M)
        nmM = small.tile([1, 1], F32, tag="stat")
        nc.vector.tensor_reduce(
            nmM[:], mstrip[:], axis=mybir.AxisListType.X, op=mybir.AluOpType.min,
        )
        dstrip = small.tile([1, NSB], F32, tag="dstrip")
        nc.vector.tensor_scalar(
            out=dstrip[:], in0=mstrip[:], scalar1=nmM[:],
            scalar2=None, op0=mybir.AluOpType.subtract,
        )
        alpha = small.tile([1, NSB], F32, tag="alpha")
        nc.scalar.activation(alpha[:], dstrip[:], Exp, scale=-1.0)
        # Z = Σ z_sb·alpha_sb ; coef_sb = alpha_sb / Z
        za = small.tile([1, NSB], F32, tag="za")
        nc.vector.tensor_mul(za[:], zstrip[:], alpha[:])
        Z = small.tile([1, 1], F32, tag="stat")
        nc.vector.tensor_reduce(
            Z[:], za[:], axis=mybir.AxisListType.X, op=mybir.AluOpType.add,
        )
        rZ = small.tile([1, 1], F32, tag="stat")
        nc.vector.reciprocal(rZ[:], Z[:])
        coef = small.tile([1, NSB], F32, tag="coef")
        nc.vector.tensor_scalar(
            out=coef[:], in0=alpha[:], scalar1=rZ[:],
            scalar2=None, op0=mybir.AluOpType.mult,
        )

        # weights out: w[sb] = e_sb * coef_sb, one DMA per block
        for sb in range(NSB):
            wsc = epool.tile([1, SBW], F32, tag="wsc", bufs=2)
            nc.vector.tensor_scalar(
                out=wsc[:], in0=e_sbs[sb][:],
                scalar1=coef[:, sb:sb + 1], scalar2=None,
                op0=mybir.AluOpType.mult,
            )
            nc.scalar.dma_start(
                out=w_out[b:b + 1, sb * SBW:(sb + 1) * SBW], in_=wsc[:])

        # context out: ctx = Σ pc_sb * coef_sb
        ctx_sb = ctxp.tile([1, H], F32, tag="ctx_sb")
        nc.vector.tensor_scalar(
            out=ctx_sb[:], in0=pcs[0][:], scalar1=coef[:, 0:1],
            scalar2=None, op0=mybir.AluOpType.mult,
        )
        for sb in range(1, NSB):
            nc.vector.scalar_tensor_tensor(
                out=ctx_sb[:], in0=pcs[sb][:], scalar=coef[:, sb:sb + 1],
                in1=ctx_sb[:], op0=mybir.AluOpType.mult, op1=mybir.AluOpType.add,
            )
        nc.scalar.dma_start(out=ctx_out[b:b + 1, :], in_=ctx_sb[:])

    # Combine for batch b is emitted after batch b+1's scores so the PE keeps
    # dense uk work while the (cheap) rescale chain for the previous batch
    # drains on DVE/ACT.
    prev = None
    for b in range(BPC):
        state = scores_phase(b)
        if prev is not None:
            combine_phase(prev[0], *prev[1])
        prev = (b, state)
    fn, args = pending_tail[0]
    fn(*args)
    pending_tail[0] = None
    combine_phase(prev[0], *prev[1])


def _build():
    if "nc" in _CACHE:
        return _CACHE["nc"]
    nc = bacc.Bacc("TRN2", target_bir_lowering=False, debug=False, num_devices=NCORES)
    with tile.TileContext(nc) as tc:
        with ExitStack() as ctx:
            _emit(nc, tc, ctx)
    nc.compile()
    _CACHE["nc"] = nc
    return nc


def kernel(query0, query1, keys, Wa_w, Wa_b, Ua_w, Ua_b, Va_w, Va_b, trace=False):
    query0 = np.asarray(query0, dtype=np.float32).reshape(B, H)
    query1 = np.asarray(query1, dtype=np.float32).reshape(B, H)
    keys = np.asarray(keys, dtype=np.float32)
    shared = {
        "wa_w": np.ascontiguousarray(Wa_w, dtype=np.float32),
        "wa_b": np.ascontiguousarray(Wa_b, dtype=np.float32),
        "ua_w": np.ascontiguousarray(Ua_w, dtype=np.float32),
        "ua_b": np.ascontiguousarray(Ua_b, dtype=np.float32),
        "va_w": np.ascontiguousarray(np.asarray(Va_w, dtype=np.float32).reshape(H)),
    }
    nc = _build()
    core_ids = list(range(NCORES))
    in_maps = []
    for c in core_ids:
        lo, hi = c * BPC, (c + 1) * BPC
        in_maps.append({
            "q0": np.ascontiguousarray(query0[lo:hi]),
            "q1": np.ascontiguousarray(query1[lo:hi]),
            "keys": np.ascontiguousarray(keys[lo:hi]),
            **shared,
        })
    try:
        res = run_bass_kernel_spmd(nc, in_maps, core_ids, trace=trace)
    except ModuleNotFoundError:
        res = run_bass_kernel_spmd(nc, in_maps, core_ids, trace=False)
    context = np.empty((B, 1, H), dtype=np.float32)
    weights = np.empty((B, 1, S), dtype=np.float32)
    for c in core_ids:
        lo, hi = c * BPC, (c + 1) * BPC
        context[lo:hi, 0, :] = res.results[c]["ctx_out"]
        weights[lo:hi, 0, :] = res.results[c]["w_out"]
    if trace:
        kernel.last_exec_time_ns = res.exec_time_ns
        kernel.last_results = res
    return (context, weights)
